# revision 58
# baseline (speedup 1.0000x reference)
"""Trainium2 Bass kernel for nn_BatchFlipLoss (NCE batch-flip loss + CE loss).

Math (validated ~1e-4 rel err vs the jax reference, tolerance 2e-2):

The reference sums BatchCriterion over 36 flip-class pairs.  For pair
(a,b) with E_ab = exp(10*G_ab), G_ab = f_a@f_b.T (f_c = features[c::8],
L2-normalized, B=512 rows each), S_ab = rowsum(E_ab), S0_aa the
diag-zeroed own-block rowsum:

  cross pair (a!=b), D = S0_aa + S_ab:
      half = 10*d - ln(D) - 1 - ln(1 - exp(10 d)/D)
  own pair:  D = 2*S0_aa + e^10:
      pair = 2*(10*d - ln(D) - 2*S0_aa/D)

(the x^2/2 series term contributes <1e-4 relative and is dropped; d is
O(N*D) and computed on host from the f32 features; CE label logits are
an O(N) host gather.)

The device produces ONLY sum-vectors of exp(10*G) blocks plus the CE
denominators rowsum(exp(predicts)).  Core c computes blocks (c, c+j
mod 8), j=0..4.  Every sum that can be expressed as a COLUMN sum is
computed by near-free PE "mini" matmuls (lhsT = a 128-col slice of the
exp'd block, rhs = ones -> out [128,1]):

  j=0 : E_aa symmetric -> rowsums == colsums -> minis only.  The
        diagonal is killed pre-exp by accumulating -40*I into the PSUM
        Gram diag via an extra identity matmul (exp(10g-400) -> 0).
  j=1..3: colsums (partner direction) via minis; own rowsums via DVE
        4x-mode tensor_scalar accum on the bf16 exp tiles.
  j=4 : both endpoints compute their orientation; each core's COLSUMS
        are the partner's rowsums -> minis only, host reroutes.
  CE  : predicts transposed on host (class dim on partitions, rows in
        the free dim, padded to [128,4,512] with -30) -> fast-exp ->
        minis accumulate over class chunks = per-row denominators.

exp engines: the hardware only lets ACT and DVE touch PSUM (GPSIMD may
not, and cannot run TensorScalar at all), so ACT runs 13 chunks (six
[128,1024] pair activations + one single) and DVE 7 chunks via the
fast-exp bit trick  bf16bits(exp(s*x)) ~ i16(128*(s*x*log2e + 127 -
c)), one tensor_scalar per chunk straight from PSUM f32 (c tuned for
zero mean bias under the DVE's round-to-nearest f32->i16 convert).

Rowsums that remain: j1's four chunks ride DMA-transposes of the exp
tiles (XBAR SBUF->SBUF) + free PE minis; j2/j3's eight chunks are DVE
4x-mode tensor_scalar accums.  A warmup matmul pins pe_busy_start ~0 so
the PE p-state ramp elapses during the DMA-in phase.  Inputs ride in
three DMAs (fq j0-block first so compute starts early); all outputs
ride in ONE [128, 40] f32 DMA.
"""

from contextlib import ExitStack

import numpy as np

FLIP = 8
B = 512
D = 128
C = 400
N = 4096
ALPHA = 0.03
NJ = 5
E10 = float(np.exp(np.float64(10.0)))

LOG2E = float(np.log2(np.e))
FE_C = 0.0575  # fast-exp bias constant (zero-mean under round f32->i16)
FE_B = 128.0 * (127.0 - FE_C)
FE_A_GRAM = 10.0 * 128.0 * LOG2E
FE_A_CE = 128.0 * LOG2E
CE_PAD = -30.0

# Exp units: ACT processes PAIRS of chunks from one [128,1024] PSUM
# half-slot (one activation, better per-instruction efficiency); Pool
# and DVE process single chunks from [128,512] quarter-slots.  Units
# listed in emission order per engine; j0 units first (they only need
# the fq j0-block DMA).  Units ~balance engine end-times; the last
# ACT/Pool units end in rowsum-free chunks where possible.
# The hardware forbids GPSIMD (Pool) from reading PSUM, so only ACT
# and DVE can run the exp stage.  ACT takes 12 chunks as 6 pairs (one
# [128,1024] activation each), DVE 8 single chunks.  Pool contributes
# SBUF-only rowsums; j1's rowsums ride DMA transposes + free PE minis.
ACT_UNITS = [
    ((0, 0), (0, 3)),
    ((1, 0), (1, 1)),
    ((1, 2), (1, 3)),
    ((3, 0), (3, 1)),
    ((3, 2), (3, 3)),
    ((4, 0), (4, 3)),
]
ACT_SINGLES = [(0, 2)]
DVE_UNITS = [(0, 1), (2, 0), (2, 1), (2, 2), (2, 3), (4, 1), (4, 2)]

# global gram emission order: (engine, unit_index)
GRAM_ORDER = [
    ("act", 0), ("acts", 0), ("dve", 0),
    ("act", 1), ("dve", 1),
    ("act", 2), ("dve", 2),
    ("act", 3), ("dve", 3),
    ("act", 4), ("dve", 4),
    ("act", 5), ("dve", 5), ("dve", 6),
]

# et tiles that get DMA-transposed so their rowsums become free PE
# minis; only early-finishing units qualify (the transpose chain costs
# ~2.4us of latency).  Pairs by ACT unit index, chunks by (j, r).
TRANSPOSE_PAIRS = [1, 2]
TRANSPOSE_CHUNKS = []
# rowsums via DVE tensor_scalar accum (Pool cannot run TensorScalar
# at all per the hardware verifier)
DVE_ROWSUMS = [
    (2, 0), (2, 1), (2, 2), (2, 3),
    (3, 0), (3, 1), (3, 2), (3, 3),
]

# PSUM accumulation groups are tracked per 2KB zero-region (a whole
# bank), so a cs column's 4 minis must be emitted CONSECUTIVELY (the
# group opens and closes atomically in the in-order PE queue).  j
# groups ordered by when their last chunk's exp completes; "rs1"/"rs2"
# = transposed-rowsum minis for j1 / j2-early.
MINI_J_ORDER = [0, 1, "ce", "rs1", 2, 3, 4]

# cs_ps PSUM column layout (all colsum-mini accumulators):
#   0:4   j0 colsums (= S0_aa, diag-killed)
#   4:16  (j-1)*4+s colsums of block (c,c+j), j=1..3
#   16:20 j4 colsums (partner's rowsums)
#   20:24 CE denominators
#   24:28 j1 rowsums via DMA-transposed tiles (col 24 + r)
CS_W = 28
OUT_W = 40  # [0:12] j2/j3 rowsums (cols 4..11 used), [12:40] cs columns

_CACHE = {}


def _build_nc(assign=None):
    import concourse.tile as tile
    from concourse import bacc, mybir

    if assign is None:
        assign = ASSIGN
    f32 = mybir.dt.float32
    bf16 = mybir.dt.bfloat16
    fp8 = mybir.dt.float8e4
    i16 = mybir.dt.int16
    AF = mybir.ActivationFunctionType
    OP = mybir.AluOpType
    DR = mybir.MatmulPerfMode.DoubleRow

    nc = bacc.Bacc("TRN2", target_bir_lowering=False, debug=False)

    fq_d = nc.dram_tensor("fq", [64, 2, NJ * B], fp8, kind="ExternalInput")
    predT_d = nc.dram_tensor("predT", [128, 4, B], bf16, kind="ExternalInput")
    out_d = nc.dram_tensor("out", [128, OUT_W], f32, kind="ExternalOutput")

    with tile.TileContext(nc) as tc, ExitStack() as ctx:
        const = ctx.enter_context(tc.tile_pool(name="const", bufs=1))
        hpool = ctx.enter_context(tc.tile_pool(name="hp", bufs=1, space="PSUM"))
        qpool = ctx.enter_context(tc.tile_pool(name="qp", bufs=1, space="PSUM"))
        cpool = ctx.enter_context(tc.tile_pool(name="cp", bufs=1, space="PSUM"))
        epool = ctx.enter_context(tc.tile_pool(name="ep", bufs=1))
        small = ctx.enter_context(tc.tile_pool(name="sm", bufs=1))

        cs_ps = cpool.tile([128, CS_W], f32)

        # ---- warmup matmul: pins pe_busy_start ~0 so the 3us p-state
        # ramp elapses while the input DMAs are in flight.  Writes into
        # a cs column whose first real mini resets it (start=True). ----
        wu = const.tile([1, 1], bf16)
        nc.vector.memset(wu[:], 0.0)
        nc.tensor.matmul(cs_ps[0:1, 23:24], wu[:], wu[:], start=True, stop=True)

        fq = const.tile([64, 2, NJ * B], fp8)
        predT = const.tile([128, 4, B], bf16)
        ones = const.tile([128, 1], bf16)
        nc.vector.memset(ones[:], 1.0)
        out_sb = small.tile([128, OUT_W], f32)
        nc.vector.memset(out_sb[:, 0:4], 0.0)  # unused legacy rowsum cols
        cebits = small.tile([128, 4, B], bf16)

        # identity / -40*identity for the PSUM diag-kill matmul,
        # generated on-device during the DMA head (Pool iota is legal)
        colidx = const.tile([128, 128], f32)
        rowidx = const.tile([128, 1], f32)
        idN = const.tile([128, 128], bf16)
        eye40n = const.tile([128, 128], bf16)
        nc.gpsimd.iota(
            colidx[:], pattern=[[1, 128]], channel_multiplier=0,
            allow_small_or_imprecise_dtypes=True,
        )
        nc.gpsimd.iota(
            rowidx[:], pattern=[[0, 1]], channel_multiplier=1,
            allow_small_or_imprecise_dtypes=True,
        )
        nc.vector.tensor_scalar(idN[:], colidx[:], rowidx[:], None, OP.is_equal)
        nc.vector.tensor_scalar(eye40n[:], idN[:], -40.0, None, OP.mult)

        # ---- input DMAs: fq j0-block first so compute starts early ----
        nc.sync.dma_start(fq[:, :, 0:B], fq_d[:, :, 0:B])
        nc.sync.dma_start(fq[:, :, B:], fq_d[:, :, B:])
        nc.sync.dma_start(predT[:], predT_d[:, :, :])

        # ---- CE fast-exp (DVE; emitted mid-stream below) ----
        def ce_ts():
            nc.vector.tensor_scalar(
                cebits[:].bitcast(i16), predT[:], FE_A_CE, FE_B, OP.mult, OP.add
            )

        def ce_minis():
            for s in range(4):
                for cc in range(4):
                    nc.tensor.matmul(
                        cs_ps[:, 20 + s : 21 + s],
                        cebits[:, cc, s * 128 : (s + 1) * 128],
                        ones[:],
                        start=(cc == 0),
                        stop=(cc == 3),
                    )

        cs_col = {0: 0, 1: 4, 2: 8, 3: 12, 4: 16}
        et = {}  # (j, r) -> (tile, col offset)

        def emit_gram(j, r, g, off):
            nc.tensor.matmul(
                g[:, off : off + 512],
                fq[:, :, r * 128 : (r + 1) * 128],
                fq[:, :, j * B : (j + 1) * B],
                start=True,
                stop=(j != 0),
                perf_mode=DR,
            )
            if j == 0:
                # kill the diagonal pre-exp: accumulate -40*I into the
                # diag 128-block of this chunk (near-free on PE)
                nc.tensor.matmul(
                    g[:, off + r * 128 : off + (r + 1) * 128],
                    idN[:],
                    eye40n[:],
                    start=False,
                    stop=True,
                )

        # ---- Gram + exp pipeline ----
        qi = [0]
        eta = {}
        ttc = {}
        for (eng_name, ui) in GRAM_ORDER:
            if eng_name == "act":
                pair = ACT_UNITS[ui]
                g = hpool.tile([128, 1024], f32, tag=f"ha{ui % 2}")
                t = epool.tile([128, 1024], bf16, tag=f"eta{ui}")
                eta[ui] = t
                for k, (j, r) in enumerate(pair):
                    emit_gram(j, r, g, k * 512)
                    et[(j, r)] = (t, k * 512)
                nc.scalar.activation(t[:], g[:], AF.Exp, bias=0.0, scale=10.0)
                if ui in TRANSPOSE_PAIRS:
                    # rowsums of this pair via DMA transpose + PE minis
                    tt = epool.tile([128, 8, 128], bf16, tag=f"tt{ui}")
                    eta[(ui, "t")] = tt
                    nc.sync.dma_start_transpose(tt[:], t[:])
                continue
            if eng_name == "acts":
                (j, r) = ACT_SINGLES[ui]
            else:
                (j, r) = DVE_UNITS[ui]
            g = qpool.tile([128, 512], f32, tag=f"q{qi[0] % 3}")
            qi[0] += 1
            t = epool.tile([128, 512], bf16, tag=f"et{j}{r}")
            emit_gram(j, r, g, 0)
            et[(j, r)] = (t, 0)
            if eng_name == "acts":
                nc.scalar.activation(t[:], g[:], AF.Exp, bias=0.0, scale=10.0)
            else:
                nc.vector.tensor_scalar(
                    t[:].bitcast(i16), g[:], FE_A_GRAM, FE_B, OP.mult, OP.add
                )
                if (j, r) in TRANSPOSE_CHUNKS:
                    tt = epool.tile([128, 4, 128], bf16, tag=f"ttc{j}{r}")
                    ttc[(j, r)] = tt
                    nc.sync.dma_start_transpose(tt[:], t[:])
                if ui == 2:
                    # predT has landed by now; CE exp slots in here
                    ce_ts()

        # ---- DVE rowsums (4x ts accum on bf16) ----
        for (j, r) in DVE_ROWSUMS:
            t, off = et[(j, r)]
            sl = t[:, off : off + 512]
            nc.vector.tensor_scalar(
                sl,
                sl,
                1.0,
                None,
                OP.mult,
                OP.add,
                accum_out=out_sb[:, (j - 1) * 4 + r : (j - 1) * 4 + r + 1],
            )

        # ---- colsum minis: emitted after all grams (they never block
        # grams in the in-order PE queue); each column's 4 minis are
        # consecutive so only one accumulation group is ever open in
        # the cs bank ----
        for item in MINI_J_ORDER:
            if item == "ce":
                ce_minis()
                continue
            if item == "rs1":
                # j1 rowsums from the transposed pair tiles: chunk (1,r)
                # sits at k-blocks 4*(r%2)..4*(r%2)+3 of transpose pair
                # 1 + r//2; its rowsum lands on 128 partitions.
                for ui, base in ((1, 0), (2, 2)):
                    tt = eta[(ui, "t")]
                    for half in range(2):
                        r = base + half
                        col = 24 + r
                        for k in range(4):
                            nc.tensor.matmul(
                                cs_ps[:, col : col + 1],
                                tt[:, half * 4 + k, :],
                                ones[:],
                                start=(k == 0),
                                stop=(k == 3),
                            )
                continue
            if item == "rs2":
                # j2 r0/r1 rowsums from transposed chunk tiles
                for (j, r), tt in ttc.items():
                    col = 28 + r
                    for k in range(4):
                        nc.tensor.matmul(
                            cs_ps[:, col : col + 1],
                            tt[:, k, :],
                            ones[:],
                            start=(k == 0),
                            stop=(k == 3),
                        )
                continue
            j = item
            for s in range(4):
                col = cs_col[j] + s
                for n, r in enumerate(range(4)):
                    t, off = et[(j, r)]
                    nc.tensor.matmul(
                        cs_ps[:, col : col + 1],
                        t[:, off + s * 128 : off + (s + 1) * 128],
                        ones[:],
                        start=(n == 0),
                        stop=(n == 3),
                    )

        # ---- outputs: rowsum columns ship early (no staging needed);
        # the staged cs columns ride the final DMA ----
        nc.sync.dma_start(out_d[:, 0:12], out_sb[:, 0:12])
        nc.vector.tensor_copy(out_sb[:, 12 : 12 + CS_W], cs_ps[:])
        nc.sync.dma_start(out_d[:, 12:], out_sb[:, 12:])

    nc.compile()
    return nc


def _get_nc(**kw):
    key = tuple(sorted(kw.items()))
    if key not in _CACHE:
        _CACHE[key] = _build_nc(**kw)
    return _CACHE[key]


def _prep_in_maps(predicts, labels, features):
    import ml_dtypes

    feats = np.ascontiguousarray(features, dtype=np.float32)
    pred = np.ascontiguousarray(predicts, dtype=np.float32)
    f8 = feats.reshape(B, FLIP, D).transpose(1, 0, 2)  # [8,512,128] f8[c]=feats[c::8]
    f8q = f8.astype(ml_dtypes.float8_e4m3)
    in_maps = []
    for a in range(FLIP):
        order = [(a + i) % FLIP for i in range(NJ)]
        fo = f8q[order]  # [5, 512, 128]
        # fq[p, i, j*512+row] = fo[j, row, i*64+p]
        fqa = np.ascontiguousarray(
            fo.reshape(NJ, B, 2, 64).transpose(3, 2, 0, 1).reshape(64, 2, NJ * B)
        )
        # predT[p, cc, row] = pred[a*512+row, cc*128+p], padded with -30
        pt = np.full((128, 4, B), CE_PAD, dtype=np.float32)
        pc = pred[a * B : (a + 1) * B]  # [512, 400]
        for cc in range(4):
            w = min(128, C - cc * 128)
            pt[:w, cc, :] = pc[:, cc * 128 : cc * 128 + w].T
        in_maps.append(
            {
                "fq": fqa,
                "predT": pt.astype(ml_dtypes.bfloat16),
            }
        )
    return in_maps


def _combine(outs, predicts, labels, features):
    """Host-side O(rows) combine."""
    feats = np.ascontiguousarray(features, dtype=np.float32)
    f8 = feats.reshape(B, FLIP, D).transpose(1, 0, 2).astype(np.float64)

    S1 = {}
    se = np.empty(N, dtype=np.float64)
    for c in range(FLIP):
        o = np.asarray(outs[c]["out"], np.float64)  # [128, 40]
        cs = o[:, 12:40]
        S1[(c, c)] = cs[:, 0:4].T.reshape(B)
        S1[(c, (c + 1) % FLIP)] = cs[:, 24:28].T.reshape(B)  # transposed rowsums
        S1[(c, (c + 2) % FLIP)] = o[:, 4:8].T.reshape(B)
        S1[(c, (c + 3) % FLIP)] = o[:, 8:12].T.reshape(B)
        for j in (1, 2, 3):
            S1[((c + j) % FLIP, c)] = cs[:, 4 + (j - 1) * 4 : 4 + j * 4].T.reshape(B)
        # j=4: my colsums are the partner's rowsums
        S1[((c + 4) % FLIP, c)] = cs[:, 16:20].T.reshape(B)
        se[c * B : (c + 1) * B] = cs[:, 20:24].T.reshape(B)

    nce = 0.0
    for a in range(FLIP):
        S10 = S1[(a, a)]
        for b in range(FLIP):
            d = np.einsum("pd,pd->p", f8[a], f8[b])
            if a == b:
                N1 = 2.0 * S10
                Dv = N1 + E10
                half = 10.0 * d - np.log(Dv) - N1 / Dv
                nce += 2.0 * half.sum()
            else:
                N1 = S10 + S1[(a, b)]
                half = (
                    10.0 * d
                    - np.log(N1)
                    - 1.0
                    - np.log1p(-np.exp(10.0 * d) / N1)
                )
                nce += half.sum()

    xlab = np.ascontiguousarray(predicts, dtype=np.float64)[
        np.arange(N), np.asarray(labels)
    ]
    ce = (np.log(se) - xlab).sum()
    val = ALPHA * (-(nce) / 1024.0) + ce / N
    return np.array(val, dtype=np.float32)


def _run_hw(in_maps, trace=False):
    from concourse.bass_utils import run_bass_kernel_spmd

    nc = _get_nc()
    res = run_bass_kernel_spmd(nc, in_maps, core_ids=list(range(FLIP)), trace=trace)
    return res


def kernel(predicts, labels, features, indexs=None, **_):
    in_maps = _prep_in_maps(predicts, labels, features)
    res = _run_hw(in_maps)
    return _combine(res.results, predicts, labels, features)


def kernel_sim(predicts, labels, features, indexs=None, **_):
    """CoreSim (CPU simulator) path for fast correctness iteration."""
    from concourse.bass_interp import CoreSim

    nc = _get_nc()
    in_maps = _prep_in_maps(predicts, labels, features)
    outs = []
    for a in range(FLIP):
        sim = CoreSim(nc, trace=False)
        for k, v in in_maps[a].items():
            sim.tensor(k)[:] = v
        sim.simulate()
        outs.append({"out": np.array(sim.tensor("out"))})
    return _combine(outs, predicts, labels, features)
